# revision 62
# baseline (speedup 1.0000x reference)
"""Multi-head causal attention (QKV proj + RoPE + softmax attention + out proj)
as a distributed Bass kernel on 8 Trainium2 NeuronCores.

Sharding: tensor-parallel over heads (2 of 16 heads per core).

Pipeline (v3): one token chunk (512) per step — QKV projection + RoPE for
the chunk, then immediately the attention q-chunk it enables (causal), so
the 4 AllGathers fire evenly through the kernel instead of bunching at the
end. Everything lives in ONE tile-pool scope (no mid-kernel pool barrier).

Layouts: Q/K d-major in SBUF; V token-major with a ones column (softmax
denominator comes free out of the PV matmul); attention output token-major,
normalized in a single DVE op; AllGather is token-major h-major rows so the
out-projection can read it back d-major via contiguous X-bar DMA-transposes
(all on ONE HWDGE queue — concurrent transposes from two queues corrupt).
Out-projection runs flipped (wo stationary, tokens moving, N=512) emitting
out^T; the host transposes and concatenates the 8 column slices.

All matmuls bf16 (fp32 PSUM); softmax without max-subtraction (scores ~N(0,1)).
"""

import math
import os as _os
import numpy as np
import ml_dtypes

B, S, D, H = 2, 2048, 2048, 16
HD = 128                  # head dim
P = 128                   # SBUF partitions
NT = B * S                # 4096 tokens
N_CORES = 8
HPC = H // N_CORES        # heads per core
DQ = HPC * HD             # 256 q/k/v rows per core
KC = D // P               # 16 contraction chunks
TCH = 512                 # token chunk
NTC = NT // TCH           # 8
SBK = S // P              # 16 key blocks per batch
QCH = 512                 # q chunk in attention
SH = S // 2               # AllGather half (1024 tokens)
BF = ml_dtypes.bfloat16

_cache = {}


def _vaug_col(b, i, h):
    return ((b * SBK + i) * HPC + h) * (HD + 1)


def _attn_col(b, i, h):
    return ((b * SBK + i) * HPC + h) * HD


def _build(mask_mode):
    from concourse import bacc
    import concourse.mybir as mybir
    import concourse.tile as tile
    from concourse.tile_rust import add_dep_helper

    bf = mybir.dt.bfloat16
    f32 = mybir.dt.float32
    EXP = mybir.ActivationFunctionType.Exp
    scale = 1.0 / math.sqrt(HD)
    causal = mask_mode == "causal"
    fullexp = _os.environ.get("KFULLEXP", "0") == "1"
    dbg = _os.environ.get("KDBG", "0") == "1"

    nc = bacc.Bacc("TRN2", target_bir_lowering=False, debug=False,
                   num_devices=N_CORES)

    xRe = nc.declare_dram_parameter("xRe", [P, NTC * KC * TCH], bf,
                                    isOutput=False)
    wqp = nc.declare_dram_parameter("wqp", [P, KC * DQ], bf, isOutput=False)
    wkp = nc.declare_dram_parameter("wkp", [P, KC * DQ], bf, isOutput=False)
    wvp = nc.declare_dram_parameter("wvp", [P, KC * DQ], bf, isOutput=False)
    wop = nc.declare_dram_parameter("wop", [P, KC * DQ], bf, isOutput=False)
    cro = nc.declare_dram_parameter("cro", [P, S], bf, isOutput=False)
    sro = nc.declare_dram_parameter("sro", [P, S], bf, isOutput=False)
    cst = nc.declare_dram_parameter("cst", [P, 3 * P], bf, isOutput=False)
    mskT = None
    if mask_mode == "general":
        mskT = nc.declare_dram_parameter("mskT", [S, S], bf, isOutput=False)
    outT = nc.declare_dram_parameter("outT", [DQ, NT], f32, isOutput=True)
    attnD = agD = qD = kD = vD = None
    if dbg:
        attnD = nc.declare_dram_parameter("attnD", [P, B * SBK * HPC * HD],
                                          bf, isOutput=True)
        agD = nc.declare_dram_parameter("agD", [N_CORES * DQ, SH], bf,
                                        isOutput=True)
        qD = nc.declare_dram_parameter("qD", [P, HPC * NT], bf, isOutput=True)
        kD = nc.declare_dram_parameter("kD", [P, HPC * NT], bf, isOutput=True)
        vD = nc.declare_dram_parameter("vD", [P, B * SBK * HPC * (HD + 1)],
                                       bf, isOutput=True)

    rg = [list(range(N_CORES))]

    with tile.TileContext(nc) as tc:
        with (
            tc.tile_pool(name="per", bufs=1) as per,
            tc.tile_pool(name="stage", bufs=3) as stage,
            tc.tile_pool(name="rt", bufs=2) as rt,
            tc.tile_pool(name="dram", bufs=1, space="DRAM") as drp,
            tc.tile_pool(name="ptp",
                         bufs=(1 if mask_mode == "general" else 2)) as ptp,
            tc.tile_pool(name="xs", bufs=5) as xs,
            tc.tile_pool(name="ags", bufs=14) as ags,
            tc.tile_pool(name="ostp", bufs=2) as ostp,
            tc.tile_pool(name="mkp", bufs=4) as mkp,
            tc.tile_pool(name="ps_qk", bufs=2, space="PSUM") as ps_qk,
            tc.tile_pool(name="ps_v", bufs=2, space="PSUM") as ps_v,
            tc.tile_pool(name="ps_st", bufs=2, space="PSUM") as ps_st,
            tc.tile_pool(name="ps_pv", bufs=2, space="PSUM") as ps_pv,
        ):
            # ---------------- persistent SBUF ----------------
            q_sb = per.tile([P, HPC * NT], bf)       # d-major Q, head h at h*NT
            k_sb = per.tile([P, HPC * NT], bf)
            vaug_sb = per.tile([P, B * SBK * HPC * (HD + 1)], bf)
            attn_sb = per.tile([P, B * SBK * HPC * HD], bf)  # token-major out
            wo_sb = per.tile([P, KC * DQ], bf)
            cst_sb = per.tile([P, 3 * P], bf)
            perm = cst_sb[:, 0:P]
            tri01 = cst_sb[:, P:2 * P]
            ident = cst_sb[:, 2 * P:3 * P]

            wq_sb = per.tile([P, KC * DQ], bf, name="wq_sb")
            wk_sb = per.tile([P, KC * DQ], bf, name="wk_sb")
            wv_sb = per.tile([P, KC * DQ], bf, name="wv_sb")
            cro_sb = per.tile([P, S], bf, name="cro_sb")
            sro_sb = per.tile([P, S], bf, name="sro_sb")

            # startup is chip-wide HBM-bound (8 cores burst at once): put
            # only what gates the first matmuls up front, defer the rest
            HW_ = KC * DQ // 2
            nc.sync.dma_start(out=cst_sb[:], in_=cst[:, :])
            nc.sync.dma_start(out=wq_sb[:, 0:HW_], in_=wqp[:, 0:HW_])
            nc.scalar.dma_start(out=wk_sb[:, 0:HW_], in_=wkp[:, 0:HW_])
            nc.sync.dma_start(out=wq_sb[:, HW_:], in_=wqp[:, HW_:])
            nc.gpsimd.dma_start(out=wv_sb[:, 0:HW_], in_=wvp[:, 0:HW_])
            nc.gpsimd.dma_start(out=wk_sb[:, HW_:], in_=wkp[:, HW_:])
            nc.gpsimd.dma_start(out=wv_sb[:, HW_:], in_=wvp[:, HW_:])
            nc.gpsimd.dma_start(out=cro_sb[:], in_=cro[:, :])
            nc.gpsimd.dma_start(out=sro_sb[:], in_=sro[:, :])
            # ones columns for the PV denominator trick
            nc.gpsimd.memset(vaug_sb[:], 1.0)

            # d-major bounce: rows (h, dd), cols = tokens of the half
            bounce = [[drp.tile([DQ, SH], bf, name=f"bounce{b}{f}",
                                tag=f"bounce{b}{f}")
                       for f in range(2)] for b in range(B)]
            ag = [[drp.tile([N_CORES * DQ, SH], bf, addr_space="Shared",
                            name=f"ag{b}{f}", tag=f"ag{b}{f}")
                   for f in range(2)] for b in range(B)]
            # the LAST AllGather is on the critical tail: split it per-qc so
            # its first half ships one token-chunk earlier
            bounce11 = [drp.tile([DQ, QCH], bf, name=f"bounce11{q}",
                                 tag=f"bounce11{q}") for q in range(2)]
            ag11 = [drp.tile([N_CORES * DQ, QCH], bf, addr_space="Shared",
                             name=f"ag11{q}", tag=f"ag11{q}")
                    for q in range(2)]

            def flush_q11(qi):
                # b=1, half=1, quarter qi: tokens [S+SH+qi*QCH, +QCH)
                for h in range(HPC):
                    c0 = h * NT + S + SH + qi * QCH
                    nc.gpsimd.dma_start(
                        out=bounce11[qi][h * HD:(h + 1) * HD, :],
                        in_=attn_sb[:, c0:c0 + QCH])
                nc.gpsimd.collective_compute(
                    "AllGather", mybir.AluOpType.bypass,
                    replica_groups=rg,
                    ins=[bounce11[qi].opt()], outs=[ag11[qi].opt()])

            def rope(ps, dst, t0b):
                # dst (bf16, [P, TCH]) = cos*z + sin*pairswap(z); tables are
                # pre-swizzled so this is cro*z + sro*zsw elementwise
                zb = stage.tile([P, TCH], bf, tag="zb", name="zb")
                nc.vector.tensor_copy(zb[:], ps[:])
                t1 = rt.tile([P, TCH], f32, tag="t1", name="t1")
                nc.vector.tensor_mul(t1[:], zb[:], cro_sb[:, t0b:t0b + TCH])
                t2 = rt.tile([P, TCH], f32, tag="t2", name="t2")
                if _os.environ.get("KSWAPDMA", "0") == "1":
                    # pairswap via HWDGE SBUF->SBUF strided DMA (frees PE)
                    zsw = stage.tile([P, TCH], bf, tag="zsw", name="zsw")
                    nc.sync.dma_start(out=zsw[:, 0:TCH:2],
                                      in_=zb[:, 1:TCH:2])
                    nc.sync.dma_start(out=zsw[:, 1:TCH:2],
                                      in_=zb[:, 0:TCH:2])
                    nc.vector.tensor_mul(t2[:], zsw[:],
                                         sro_sb[:, t0b:t0b + TCH])
                else:
                    # pairswap via PE permutation matmul (DVE strided-copy
                    # swap mis-executes in situ despite passing in isolation)
                    zs = ps_st.tile([P, TCH], f32, tag="st", name="zs")
                    nc.tensor.matmul(zs[:], perm, zb[:])
                    nc.vector.tensor_mul(t2[:], zs[:],
                                         sro_sb[:, t0b:t0b + TCH])
                nc.vector.tensor_add(dst, t1[:], t2[:])

            def attn_chunk(b, qc):
                # attention for q tokens [qc*512, (qc+1)*512) of batch b
                for h in range(HPC):
                    qoff = h * NT + b * S
                    n_s = SBK if not causal else 4 * qc + 4
                    pt = ptp.tile([P, SBK * QCH], bf, tag="pt",
                                  name=f"pt{b}{h}{qc}")
                    for sb in range(n_s):
                        stp = ps_st.tile([P, QCH], f32, tag="st",
                                         name=f"st{b}{h}{qc}{sb}")
                        nc.tensor.matmul(
                            stp[:],
                            k_sb[:, qoff + sb * P:qoff + (sb + 1) * P],
                            q_sb[:, qoff + qc * QCH:qoff + (qc + 1) * QCH])
                        if mask_mode == "general":
                            mk = mkp.tile([P, QCH], bf, tag="mk",
                                          name=f"mk{b}{h}{qc}{sb}")
                            nc.sync.dma_start(
                                out=mk[:],
                                in_=mskT[sb * P:(sb + 1) * P,
                                         qc * QCH:(qc + 1) * QCH])
                            nc.vector.tensor_add(stp[:], stp[:], mk[:])
                        off = (sb - 4 * qc) * P \
                            if (causal and not fullexp and sb > 4 * qc) else 0
                        nc.scalar.activation(
                            pt[:, sb * QCH + off:(sb + 1) * QCH],
                            stp[:, off:QCH], EXP, scale=scale)
                    if causal:
                        for j in range(QCH // P):
                            sb = 4 * qc + j
                            c0 = sb * QCH + j * P
                            nc.vector.tensor_mul(
                                pt[:, c0:c0 + P], pt[:, c0:c0 + P], tri01)
                    for jj in range(QCH // P):
                        qb = 4 * qc + jj
                        n_pv = SBK if not causal else qb + 1
                        pv = ps_pv.tile([P, HD + 1], f32, tag="pv",
                                        name=f"pv{b}{h}{qb}")
                        for sb in range(n_pv):
                            nc.tensor.matmul(
                                pv[:],
                                pt[:, sb * QCH + jj * P:
                                   sb * QCH + (jj + 1) * P],
                                vaug_sb[:, _vaug_col(b, sb, h):
                                        _vaug_col(b, sb, h) + HD + 1],
                                start=(sb == 0), stop=(sb == n_pv - 1))
                        rec = stage.tile([P, 1], f32, tag="rec",
                                         name=f"rec{b}{h}{qb}")
                        nc.vector.reciprocal(rec[:], pv[:, HD:HD + 1])
                        ast = stage.tile([P, P], bf, tag="ast",
                                         name=f"ast{b}{h}{qb}")
                        nc.vector.tensor_scalar_mul(ast[:], pv[:, 0:HD],
                                                    rec[:])
                        trp = ps_pv.tile([P, P], bf, tag="pv",
                                         name=f"tr{b}{h}{qb}")
                        nc.tensor.transpose(trp[:], ast[:], ident)
                        nc.vector.tensor_copy(
                            attn_sb[:, h * NT + b * S + qb * P:
                                    h * NT + b * S + (qb + 1) * P],
                            trp[:])

            def flush_half(b, half):
                # d-major bounce out, then AllGather
                for h in range(HPC):
                    c0 = h * NT + b * S + half * SH
                    nc.gpsimd.dma_start(
                        out=bounce[b][half][h * HD:(h + 1) * HD, :],
                        in_=attn_sb[:, c0:c0 + SH])
                nc.gpsimd.collective_compute(
                    "AllGather", mybir.AluOpType.bypass,
                    replica_groups=rg,
                    ins=[bounce[b][half].opt()], outs=[ag[b][half].opt()])

            def outproj(b, half, split=False):
                agts = []
                for kk in range(KC):
                    if split:
                        pair = []
                        for qi in range(2):
                            agt = ags.tile([P, QCH], bf, tag="agt",
                                           name=f"agt{b}{half}{kk}{qi}")
                            nc.sync.dma_start(
                                out=agt[:],
                                in_=ag11[qi][kk * P:(kk + 1) * P, :])
                            pair.append(agt)
                        agts.append(pair)
                    else:
                        agt = ags.tile([P, SH], bf, tag="agt",
                                       name=f"agt{b}{half}{kk}")
                        nc.sync.dma_start(
                            out=agt[:],
                            in_=ag[b][half][kk * P:(kk + 1) * P, :])
                        agts.append(agt)
                # kk-outer: each agt tile's last read happens at its own kk
                # step, so the ags ring drains in load order (no stalls)
                ops = {}
                for tg in range(2):
                    pool = ps_qk if tg == 0 else ps_v
                    tag = "qkps" if tg == 0 else "vps"
                    for oc in range(2):
                        ops[tg, oc] = pool.tile(
                            [P, QCH], f32, tag=tag,
                            name=f"op{b}{half}{tg}{oc}")
                if split:
                    # tg-outer so tg0 (first AllGather piece) completes
                    # without waiting for the second piece
                    for tg in range(2):
                        for kk in range(KC):
                            for oc in range(2):
                                nc.tensor.matmul(
                                    ops[tg, oc],
                                    wo_sb[:, kk * DQ + oc * P:
                                          kk * DQ + (oc + 1) * P],
                                    agts[kk][tg][:],
                                    start=(kk == 0), stop=(kk == KC - 1))
                else:
                    for kk in range(KC):
                        for tg in range(2):
                            mv = agts[kk][:, tg * QCH:(tg + 1) * QCH]
                            for oc in range(2):
                                nc.tensor.matmul(
                                    ops[tg, oc],
                                    wo_sb[:, kk * DQ + oc * P:
                                          kk * DQ + (oc + 1) * P],
                                    mv,
                                    start=(kk == 0), stop=(kk == KC - 1))
                for tg in range(2):
                    t0 = b * S + half * SH + tg * QCH
                    for oc in range(2):
                        ost = ostp.tile([P, QCH], f32, tag="ost",
                                        name=f"ost{b}{half}{tg}{oc}")
                        nc.scalar.activation(
                            ost[:], ops[tg, oc][:],
                            mybir.ActivationFunctionType.Copy)
                        nc.gpsimd.dma_start(
                            out=outT[oc * P:(oc + 1) * P, t0:t0 + QCH],
                            in_=ost[:])

            # ---------------- main pipeline ----------------
            gate_mm = [None] * NTC
            for tci in range(NTC):
                t0 = tci * TCH
                b = tci // (NTC // B)
                lc = tci % (NTC // B)      # chunk index within batch
                t0b = lc * TCH             # within-batch token offset
                x_t = []
                for xh in range(4):
                    xt_h = xs.tile([P, KC * TCH // 4], bf, tag="xt",
                                   name=f"xt{tci}_{xh}")
                    nc.scalar.dma_start(
                        out=xt_h[:],
                        in_=xRe[:, (tci * KC + xh * KC // 4) * TCH:
                                (tci * KC + (xh + 1) * KC // 4) * TCH])
                    x_t.append(xt_h)

                vp = [ps_v.tile([P, 2 * DQ], f32, tag="vps",
                                name=f"vp{tci}_{u}") for u in range(2)]
                for m in range(HPC):
                    qp = ps_qk.tile([P, TCH], f32, tag="qkps",
                                    name=f"qp{tci}_{m}")
                    kp = ps_qk.tile([P, TCH], f32, tag="qkps",
                                    name=f"kp{tci}_{m}")
                    vfirst = {}
                    for kk in range(KC):
                        xth = x_t[kk // (KC // 4)]
                        kkl = kk % (KC // 4)
                        xt = xth[:, kkl * TCH:(kkl + 1) * TCH]
                        st = (kk == 0)
                        sp = (kk == KC - 1)
                        nc.tensor.matmul(
                            qp[:],
                            wq_sb[:, kk * DQ + m * HD:kk * DQ + (m + 1) * HD],
                            xt, start=st, stop=sp)
                        kmm = nc.tensor.matmul(
                            kp[:],
                            wk_sb[:, kk * DQ + m * HD:kk * DQ + (m + 1) * HD],
                            xt, start=st, stop=sp)
                        if m == HPC - 1 and sp:
                            gate_mm[tci] = kmm
                        if m == 0:
                            for tb in range(TCH // P):
                                mm = nc.tensor.matmul(
                                    vp[tb // 2][:, (tb % 2) * DQ:
                                                (tb % 2 + 1) * DQ],
                                    xth[:, kkl * TCH + tb * P:
                                        kkl * TCH + (tb + 1) * P],
                                    wv_sb[:, kk * DQ:(kk + 1) * DQ],
                                    start=(st and tb % 2 == 0), stop=sp,
                                    skip_group_check=(tb % 2 == 1))
                                if kk == 0:
                                    vfirst[tb] = mm
                    if m == 0:
                        for u in range(2):
                            add_dep_helper(vfirst[u * 2 + 1].ins,
                                           vfirst[u * 2].ins, sync=False,
                                           reason="bank-clear 2nd V group")
                    rope(qp, q_sb[:, m * NT + t0:m * NT + t0 + TCH], t0b)
                    rope(kp, k_sb[:, m * NT + t0:m * NT + t0 + TCH], t0b)
                # V psum -> vaug (token-major, per head)
                for tb in range(TCH // P):
                    i = (t0b + tb * P) // P
                    for h in range(HPC):
                        c0 = _vaug_col(b, i, h)
                        nc.vector.tensor_copy(
                            vaug_sb[:, c0:c0 + HD],
                            vp[tb // 2][:, (tb % 2) * DQ + h * HD:
                                        (tb % 2) * DQ + (h + 1) * HD])

                if causal:
                    attn_chunk(b, lc)
                    if b == 1 and lc >= 2:
                        flush_q11(lc - 2)
                    elif lc % 2 == 1:
                        flush_half(b, lc // 2)
                    if tci == 2:
                        wo_dma = nc.sync.dma_start(out=wo_sb[:],
                                                   in_=wop[:, :])
                        add_dep_helper(wo_dma.ins, gate_mm[1].ins,
                                       sync=True, reason="defer wo load")
                elif not causal and lc == (NTC // B) - 1:
                    # non-causal needs the batch's full K/V first
                    for qc in range(4):
                        attn_chunk(b, qc)
                    flush_half(b, 0)
                    if b == 1:
                        flush_q11(0)
                        flush_q11(1)
                    else:
                        flush_half(b, 1)
                    if tci == 3:
                        wo_dma = nc.sync.dma_start(out=wo_sb[:],
                                                   in_=wop[:, :])
                        add_dep_helper(wo_dma.ins, gate_mm[1].ins,
                                       sync=True, reason="defer wo load")

            outproj(0, 0)
            outproj(0, 1)
            outproj(1, 0)
            outproj(1, 1, split=True)
            if dbg:
                nc.sync.dma_start(out=attnD[:, :], in_=attn_sb[:])
                nc.gpsimd.dma_start(out=agD[:, :], in_=ag[0][1][:, :])
                nc.sync.dma_start(out=qD[:, :], in_=q_sb[:])
                nc.sync.dma_start(out=kD[:, :], in_=k_sb[:])
                nc.sync.dma_start(out=vD[:, :], in_=vaug_sb[:])

    nc.compile()
    return nc


def _host_prep(inputs):
    x = np.ascontiguousarray(np.asarray(inputs["x"], np.float32).reshape(NT, D))
    wq = np.asarray(inputs["wq"], np.float32)
    wk = np.asarray(inputs["wk"], np.float32)
    wv = np.asarray(inputs["wv"], np.float32)
    wo = np.asarray(inputs["wo"], np.float32)
    cos = np.asarray(inputs["freqs_cos"], np.float32)
    sin = np.asarray(inputs["freqs_sin"], np.float32)
    mask = np.asarray(inputs["mask"], np.float32).reshape(S, S)

    tril = np.tril(np.ones((S, S), bool))
    if not mask.any():
        mode = "zeros"
    elif (mask[tril] == 0).all() and (mask[~tril] <= -1e8).all():
        mode = "causal"
    else:
        mode = "general"

    # x packed tci-major: xRe[p, (tci*KC+kk)*TCH + t] = x[tci*TCH+t, kk*P+p]
    xRe = np.ascontiguousarray(
        x.reshape(NTC, TCH, KC, P).transpose(3, 0, 2, 1)
        .reshape(P, NTC * KC * TCH).astype(BF))

    C = np.empty((P, S), np.float32)
    Sn = np.empty((P, S), np.float32)
    C[0::2] = cos.T
    C[1::2] = cos.T
    Sn[0::2] = -sin.T
    Sn[1::2] = sin.T
    cro = np.ascontiguousarray(C.astype(BF))
    sro = np.ascontiguousarray(Sn.astype(BF))
    cst = np.zeros((P, 3 * P), np.float32)
    pr = np.zeros((P, P), np.float32)
    idx = np.arange(0, P, 2)
    pr[idx, idx + 1] = 1.0
    pr[idx + 1, idx] = 1.0
    cst[:, 0:P] = pr
    cst[:, P:2 * P] = np.triu(np.ones((P, P), np.float32))
    cst[:, 2 * P:3 * P] = np.eye(P)
    cst = np.ascontiguousarray(cst.astype(BF))

    def wpack(w, r):
        wT = np.ascontiguousarray(w[r, :].T)
        return np.ascontiguousarray(
            wT.reshape(KC, P, DQ).transpose(1, 0, 2)
            .reshape(P, KC * DQ).astype(BF))

    in_maps = []
    for c in range(N_CORES):
        r = slice(c * DQ, (c + 1) * DQ)
        m = {
            "xRe": xRe,
            "wqp": wpack(wq, r),
            "wkp": wpack(wk, r),
            "wvp": wpack(wv, r),
            "wop": wpack(wo, r),
            "cro": cro,
            "sro": sro,
            "cst": cst,
        }
        if mode == "general":
            m["mskT"] = np.ascontiguousarray(
                (mask.T * math.sqrt(HD)).astype(BF))
        in_maps.append(m)
    return mode, in_maps


LAST_RESULT = None


def kernel(**inputs):
    global LAST_RESULT
    from concourse.bass_utils import run_bass_kernel_spmd

    mode, in_maps = _host_prep(inputs)
    if mode not in _cache:
        _cache[mode] = _build(mode)
    nc = _cache[mode]

    res = run_bass_kernel_spmd(nc, in_maps, list(range(N_CORES)))
    LAST_RESULT = res

    out_full = np.empty((NT, D), np.float32)
    for c in range(N_CORES):
        out_full[:, c * DQ:(c + 1) * DQ] = res.results[c]["outT"].T
    return out_full.reshape(B, S, D)


# revision 64
# speedup vs baseline: 1.0439x; 1.0439x over previous
"""Multi-head causal attention (QKV proj + RoPE + softmax attention + out proj)
as a distributed Bass kernel on 8 Trainium2 NeuronCores.

Sharding: tensor-parallel over heads (2 of 16 heads per core).

Pipeline (v3): one token chunk (512) per step — QKV projection + RoPE for
the chunk, then immediately the attention q-chunk it enables (causal), so
the 4 AllGathers fire evenly through the kernel instead of bunching at the
end. Everything lives in ONE tile-pool scope (no mid-kernel pool barrier).

Layouts: Q/K d-major in SBUF; V token-major with a ones column (softmax
denominator comes free out of the PV matmul); attention output token-major,
normalized in a single DVE op; AllGather is token-major h-major rows so the
out-projection can read it back d-major via contiguous X-bar DMA-transposes
(all on ONE HWDGE queue — concurrent transposes from two queues corrupt).
Out-projection runs flipped (wo stationary, tokens moving, N=512) emitting
out^T; the host transposes and concatenates the 8 column slices.

All matmuls bf16 (fp32 PSUM); softmax without max-subtraction (scores ~N(0,1)).
"""

import math
import os as _os
import numpy as np
import ml_dtypes

B, S, D, H = 2, 2048, 2048, 16
HD = 128                  # head dim
P = 128                   # SBUF partitions
NT = B * S                # 4096 tokens
N_CORES = 8
HPC = H // N_CORES        # heads per core
DQ = HPC * HD             # 256 q/k/v rows per core
KC = D // P               # 16 contraction chunks
TCH = 512                 # token chunk
NTC = NT // TCH           # 8
SBK = S // P              # 16 key blocks per batch
QCH = 512                 # q chunk in attention
SH = S // 2               # AllGather half (1024 tokens)
BF = ml_dtypes.bfloat16

_cache = {}


def _vaug_col(b, i, h):
    return ((b * SBK + i) * HPC + h) * (HD + 1)


def _attn_col(b, i, h):
    return ((b * SBK + i) * HPC + h) * HD


def _build(mask_mode):
    from concourse import bacc
    import concourse.mybir as mybir
    import concourse.tile as tile
    from concourse.tile_rust import add_dep_helper

    bf = mybir.dt.bfloat16
    f32 = mybir.dt.float32
    EXP = mybir.ActivationFunctionType.Exp
    scale = 1.0 / math.sqrt(HD)
    causal = mask_mode == "causal"
    fullexp = _os.environ.get("KFULLEXP", "0") == "1"
    dbg = _os.environ.get("KDBG", "0") == "1"

    nc = bacc.Bacc("TRN2", target_bir_lowering=False, debug=False,
                   num_devices=N_CORES)

    xRe = nc.declare_dram_parameter("xRe", [P, NTC * KC * TCH], bf,
                                    isOutput=False)
    wqp = nc.declare_dram_parameter("wqp", [P, KC * DQ], bf, isOutput=False)
    wkp = nc.declare_dram_parameter("wkp", [P, KC * DQ], bf, isOutput=False)
    wvp = nc.declare_dram_parameter("wvp", [P, KC * DQ], bf, isOutput=False)
    wop = nc.declare_dram_parameter("wop", [P, KC * DQ], bf, isOutput=False)
    cro = nc.declare_dram_parameter("cro", [P, S], bf, isOutput=False)
    sro = nc.declare_dram_parameter("sro", [P, S], bf, isOutput=False)
    cst = nc.declare_dram_parameter("cst", [P, 3 * P], bf, isOutput=False)
    mskT = None
    if mask_mode == "general":
        mskT = nc.declare_dram_parameter("mskT", [S, S], bf, isOutput=False)
    outT = nc.declare_dram_parameter("outT", [DQ, NT], f32, isOutput=True)
    attnD = agD = qD = kD = vD = None
    if dbg:
        attnD = nc.declare_dram_parameter("attnD", [P, B * SBK * HPC * HD],
                                          bf, isOutput=True)
        agD = nc.declare_dram_parameter("agD", [N_CORES * DQ, SH], bf,
                                        isOutput=True)
        qD = nc.declare_dram_parameter("qD", [P, HPC * NT], bf, isOutput=True)
        kD = nc.declare_dram_parameter("kD", [P, HPC * NT], bf, isOutput=True)
        vD = nc.declare_dram_parameter("vD", [P, B * SBK * HPC * (HD + 1)],
                                       bf, isOutput=True)

    rg = [list(range(N_CORES))]

    with tile.TileContext(nc) as tc:
        with (
            tc.tile_pool(name="per", bufs=1) as per,
            tc.tile_pool(name="stage", bufs=3) as stage,
            tc.tile_pool(name="rt", bufs=2) as rt,
            tc.tile_pool(name="dram", bufs=1, space="DRAM") as drp,
            tc.tile_pool(name="ptp",
                         bufs=(1 if mask_mode == "general" else 2)) as ptp,
            tc.tile_pool(name="xs", bufs=5) as xs,
            tc.tile_pool(name="ags", bufs=14) as ags,
            tc.tile_pool(name="ostp", bufs=2) as ostp,
            tc.tile_pool(name="mkp", bufs=4) as mkp,
            tc.tile_pool(name="ps_qk", bufs=2, space="PSUM") as ps_qk,
            tc.tile_pool(name="ps_v", bufs=2, space="PSUM") as ps_v,
            tc.tile_pool(name="ps_st", bufs=2, space="PSUM") as ps_st,
            tc.tile_pool(name="ps_pv", bufs=2, space="PSUM") as ps_pv,
        ):
            # ---------------- persistent SBUF ----------------
            q_sb = per.tile([P, HPC * NT], bf)       # d-major Q, head h at h*NT
            k_sb = per.tile([P, HPC * NT], bf)
            vaug_sb = per.tile([P, B * SBK * HPC * (HD + 1)], bf)
            attn_sb = per.tile([P, B * SBK * HPC * HD], bf)  # token-major out
            wo_sb = per.tile([P, KC * DQ], bf)
            cst_sb = per.tile([P, 3 * P], bf)
            perm = cst_sb[:, 0:P]
            tri01 = cst_sb[:, P:2 * P]
            ident = cst_sb[:, 2 * P:3 * P]

            wq_sb = per.tile([P, KC * DQ], bf, name="wq_sb")
            wk_sb = per.tile([P, KC * DQ], bf, name="wk_sb")
            wv_sb = per.tile([P, KC * DQ], bf, name="wv_sb")
            cro_sb = per.tile([P, S], bf, name="cro_sb")
            sro_sb = per.tile([P, S], bf, name="sro_sb")

            # startup is chip-wide HBM-bound (8 cores burst at once): put
            # only what gates the first matmuls up front, defer the rest
            HW_ = KC * DQ // 2
            nc.sync.dma_start(out=cst_sb[:], in_=cst[:, :])
            nc.sync.dma_start(out=wq_sb[:, 0:HW_], in_=wqp[:, 0:HW_])
            nc.scalar.dma_start(out=wk_sb[:, 0:HW_], in_=wkp[:, 0:HW_])
            nc.sync.dma_start(out=wq_sb[:, HW_:], in_=wqp[:, HW_:])
            nc.gpsimd.dma_start(out=wv_sb[:, 0:HW_], in_=wvp[:, 0:HW_])
            nc.gpsimd.dma_start(out=wk_sb[:, HW_:], in_=wkp[:, HW_:])
            nc.gpsimd.dma_start(out=wv_sb[:, HW_:], in_=wvp[:, HW_:])
            nc.gpsimd.dma_start(out=cro_sb[:], in_=cro[:, :])
            nc.gpsimd.dma_start(out=sro_sb[:], in_=sro[:, :])
            # ones columns for the PV denominator trick
            nc.gpsimd.memset(vaug_sb[:], 1.0)

            # d-major bounce: rows (h, dd), cols = tokens of the half
            bounce = [[drp.tile([DQ, SH], bf, name=f"bounce{b}{f}",
                                tag=f"bounce{b}{f}")
                       for f in range(2)] for b in range(B)]
            ag = [[drp.tile([N_CORES * DQ, SH], bf, addr_space="Shared",
                            name=f"ag{b}{f}", tag=f"ag{b}{f}")
                   for f in range(2)] for b in range(B)]
            # the LAST AllGather is on the critical tail: split it per-qc so
            # its first half ships one token-chunk earlier
            bounce11 = [drp.tile([DQ, QCH], bf, name=f"bounce11{q}",
                                 tag=f"bounce11{q}") for q in range(2)]
            ag11 = [drp.tile([N_CORES * DQ, QCH], bf, addr_space="Shared",
                             name=f"ag11{q}", tag=f"ag11{q}")
                    for q in range(2)]

            def flush_q11(qi):
                # b=1, half=1, quarter qi: tokens [S+SH+qi*QCH, +QCH)
                for h in range(HPC):
                    c0 = h * NT + S + SH + qi * QCH
                    nc.gpsimd.dma_start(
                        out=bounce11[qi][h * HD:(h + 1) * HD, :],
                        in_=attn_sb[:, c0:c0 + QCH])
                nc.gpsimd.collective_compute(
                    "AllGather", mybir.AluOpType.bypass,
                    replica_groups=rg,
                    ins=[bounce11[qi].opt()], outs=[ag11[qi].opt()])

            def rope(ps, dst, t0b):
                # dst (bf16, [P, TCH]) = cos*z + sin*pairswap(z); tables are
                # pre-swizzled so this is cro*z + sro*zsw elementwise
                zb = stage.tile([P, TCH], bf, tag="zb", name="zb")
                nc.vector.tensor_copy(zb[:], ps[:])
                t1 = rt.tile([P, TCH], f32, tag="t1", name="t1")
                nc.vector.tensor_mul(t1[:], zb[:], cro_sb[:, t0b:t0b + TCH])
                t2 = rt.tile([P, TCH], f32, tag="t2", name="t2")
                if _os.environ.get("KSWAPDMA", "0") == "1":
                    # pairswap via HWDGE SBUF->SBUF strided DMA (frees PE)
                    zsw = stage.tile([P, TCH], bf, tag="zsw", name="zsw")
                    nc.sync.dma_start(out=zsw[:, 0:TCH:2],
                                      in_=zb[:, 1:TCH:2])
                    nc.sync.dma_start(out=zsw[:, 1:TCH:2],
                                      in_=zb[:, 0:TCH:2])
                    nc.vector.tensor_mul(t2[:], zsw[:],
                                         sro_sb[:, t0b:t0b + TCH])
                else:
                    # pairswap via PE permutation matmul (DVE strided-copy
                    # swap mis-executes in situ despite passing in isolation)
                    zs = ps_st.tile([P, TCH], f32, tag="st", name="zs")
                    nc.tensor.matmul(zs[:], perm, zb[:])
                    nc.vector.tensor_mul(t2[:], zs[:],
                                         sro_sb[:, t0b:t0b + TCH])
                nc.vector.tensor_add(dst, t1[:], t2[:])

            def attn_chunk(b, qc):
                # attention for q tokens [qc*512, (qc+1)*512) of batch b
                for h in range(HPC):
                    qoff = h * NT + b * S
                    n_s = SBK if not causal else 4 * qc + 4
                    pt = ptp.tile([P, SBK * QCH], bf, tag="pt",
                                  name=f"pt{b}{h}{qc}")
                    for sb in range(n_s):
                        stp = ps_st.tile([P, QCH], f32, tag="st",
                                         name=f"st{b}{h}{qc}{sb}")
                        nc.tensor.matmul(
                            stp[:],
                            k_sb[:, qoff + sb * P:qoff + (sb + 1) * P],
                            q_sb[:, qoff + qc * QCH:qoff + (qc + 1) * QCH])
                        if mask_mode == "general":
                            mk = mkp.tile([P, QCH], bf, tag="mk",
                                          name=f"mk{b}{h}{qc}{sb}")
                            nc.sync.dma_start(
                                out=mk[:],
                                in_=mskT[sb * P:(sb + 1) * P,
                                         qc * QCH:(qc + 1) * QCH])
                            nc.vector.tensor_add(stp[:], stp[:], mk[:])
                        off = (sb - 4 * qc) * P \
                            if (causal and not fullexp and sb > 4 * qc) else 0
                        nc.scalar.activation(
                            pt[:, sb * QCH + off:(sb + 1) * QCH],
                            stp[:, off:QCH], EXP, scale=scale)
                    if causal:
                        for j in range(QCH // P):
                            sb = 4 * qc + j
                            c0 = sb * QCH + j * P
                            nc.vector.tensor_mul(
                                pt[:, c0:c0 + P], pt[:, c0:c0 + P], tri01)
                    for jj in range(QCH // P):
                        qb = 4 * qc + jj
                        n_pv = SBK if not causal else qb + 1
                        pv = ps_pv.tile([P, HD + 1], f32, tag="pv",
                                        name=f"pv{b}{h}{qb}")
                        for sb in range(n_pv):
                            nc.tensor.matmul(
                                pv[:],
                                pt[:, sb * QCH + jj * P:
                                   sb * QCH + (jj + 1) * P],
                                vaug_sb[:, _vaug_col(b, sb, h):
                                        _vaug_col(b, sb, h) + HD + 1],
                                start=(sb == 0), stop=(sb == n_pv - 1))
                        rec = stage.tile([P, 1], f32, tag="rec",
                                         name=f"rec{b}{h}{qb}")
                        nc.vector.reciprocal(rec[:], pv[:, HD:HD + 1])
                        ast = stage.tile([P, P], bf, tag="ast",
                                         name=f"ast{b}{h}{qb}")
                        nc.vector.tensor_scalar_mul(ast[:], pv[:, 0:HD],
                                                    rec[:])
                        trp = ps_pv.tile([P, P], bf, tag="pv",
                                         name=f"tr{b}{h}{qb}")
                        nc.tensor.transpose(trp[:], ast[:], ident)
                        nc.vector.tensor_copy(
                            attn_sb[:, h * NT + b * S + qb * P:
                                    h * NT + b * S + (qb + 1) * P],
                            trp[:])

            def flush_half(b, half):
                # d-major bounce out, then AllGather
                for h in range(HPC):
                    c0 = h * NT + b * S + half * SH
                    nc.gpsimd.dma_start(
                        out=bounce[b][half][h * HD:(h + 1) * HD, :],
                        in_=attn_sb[:, c0:c0 + SH])
                nc.gpsimd.collective_compute(
                    "AllGather", mybir.AluOpType.bypass,
                    replica_groups=rg,
                    ins=[bounce[b][half].opt()], outs=[ag[b][half].opt()])

            def outproj(b, half, split=False):
                agts = []
                if split:
                    agts = [[None, None] for _ in range(KC)]
                    for qi in range(2):
                        for kk in range(KC):
                            agt = ags.tile([P, QCH], bf, tag="agt",
                                           name=f"agt{b}{half}{kk}{qi}")
                            nc.sync.dma_start(
                                out=agt[:],
                                in_=ag11[qi][kk * P:(kk + 1) * P, :])
                            agts[kk][qi] = agt
                for kk in range(KC):
                    if split:
                        pass
                    else:
                        agt = ags.tile([P, SH], bf, tag="agt",
                                       name=f"agt{b}{half}{kk}")
                        nc.sync.dma_start(
                            out=agt[:],
                            in_=ag[b][half][kk * P:(kk + 1) * P, :])
                        agts.append(agt)
                # kk-outer: each agt tile's last read happens at its own kk
                # step, so the ags ring drains in load order (no stalls)
                ops = {}
                for tg in range(2):
                    pool = ps_qk if tg == 0 else ps_v
                    tag = "qkps" if tg == 0 else "vps"
                    for oc in range(2):
                        ops[tg, oc] = pool.tile(
                            [P, QCH], f32, tag=tag,
                            name=f"op{b}{half}{tg}{oc}")
                for kk in range(KC):
                    for tg in range(2):
                        mv = agts[kk][tg][:] if split \
                            else agts[kk][:, tg * QCH:(tg + 1) * QCH]
                        for oc in range(2):
                            nc.tensor.matmul(
                                ops[tg, oc],
                                wo_sb[:, kk * DQ + oc * P:
                                      kk * DQ + (oc + 1) * P],
                                mv,
                                start=(kk == 0), stop=(kk == KC - 1))
                for tg in range(2):
                    t0 = b * S + half * SH + tg * QCH
                    for oc in range(2):
                        ost = ostp.tile([P, QCH], f32, tag="ost",
                                        name=f"ost{b}{half}{tg}{oc}")
                        nc.scalar.activation(
                            ost[:], ops[tg, oc][:],
                            mybir.ActivationFunctionType.Copy)
                        nc.gpsimd.dma_start(
                            out=outT[oc * P:(oc + 1) * P, t0:t0 + QCH],
                            in_=ost[:])

            # ---------------- main pipeline ----------------
            gate_mm = [None] * NTC
            for tci in range(NTC):
                t0 = tci * TCH
                b = tci // (NTC // B)
                lc = tci % (NTC // B)      # chunk index within batch
                t0b = lc * TCH             # within-batch token offset
                x_t = []
                for xh in range(4):
                    xt_h = xs.tile([P, KC * TCH // 4], bf, tag="xt",
                                   name=f"xt{tci}_{xh}")
                    nc.scalar.dma_start(
                        out=xt_h[:],
                        in_=xRe[:, (tci * KC + xh * KC // 4) * TCH:
                                (tci * KC + (xh + 1) * KC // 4) * TCH])
                    x_t.append(xt_h)

                vp = [ps_v.tile([P, 2 * DQ], f32, tag="vps",
                                name=f"vp{tci}_{u}") for u in range(2)]
                for m in range(HPC):
                    qp = ps_qk.tile([P, TCH], f32, tag="qkps",
                                    name=f"qp{tci}_{m}")
                    kp = ps_qk.tile([P, TCH], f32, tag="qkps",
                                    name=f"kp{tci}_{m}")
                    vfirst = {}
                    for kk in range(KC):
                        xth = x_t[kk // (KC // 4)]
                        kkl = kk % (KC // 4)
                        xt = xth[:, kkl * TCH:(kkl + 1) * TCH]
                        st = (kk == 0)
                        sp = (kk == KC - 1)
                        nc.tensor.matmul(
                            qp[:],
                            wq_sb[:, kk * DQ + m * HD:kk * DQ + (m + 1) * HD],
                            xt, start=st, stop=sp)
                        kmm = nc.tensor.matmul(
                            kp[:],
                            wk_sb[:, kk * DQ + m * HD:kk * DQ + (m + 1) * HD],
                            xt, start=st, stop=sp)
                        if m == HPC - 1 and sp:
                            gate_mm[tci] = kmm
                        if m == 0:
                            for tb in range(TCH // P):
                                mm = nc.tensor.matmul(
                                    vp[tb // 2][:, (tb % 2) * DQ:
                                                (tb % 2 + 1) * DQ],
                                    xth[:, kkl * TCH + tb * P:
                                        kkl * TCH + (tb + 1) * P],
                                    wv_sb[:, kk * DQ:(kk + 1) * DQ],
                                    start=(st and tb % 2 == 0), stop=sp,
                                    skip_group_check=(tb % 2 == 1))
                                if kk == 0:
                                    vfirst[tb] = mm
                    if m == 0:
                        for u in range(2):
                            add_dep_helper(vfirst[u * 2 + 1].ins,
                                           vfirst[u * 2].ins, sync=False,
                                           reason="bank-clear 2nd V group")
                    rope(qp, q_sb[:, m * NT + t0:m * NT + t0 + TCH], t0b)
                    rope(kp, k_sb[:, m * NT + t0:m * NT + t0 + TCH], t0b)
                # V psum -> vaug (token-major, per head)
                for tb in range(TCH // P):
                    i = (t0b + tb * P) // P
                    for h in range(HPC):
                        c0 = _vaug_col(b, i, h)
                        nc.vector.tensor_copy(
                            vaug_sb[:, c0:c0 + HD],
                            vp[tb // 2][:, (tb % 2) * DQ + h * HD:
                                        (tb % 2) * DQ + (h + 1) * HD])

                if causal:
                    attn_chunk(b, lc)
                    if b == 1 and lc >= 2:
                        flush_q11(lc - 2)
                    elif lc % 2 == 1:
                        flush_half(b, lc // 2)
                    if tci == 2:
                        wo_dma = nc.sync.dma_start(out=wo_sb[:],
                                                   in_=wop[:, :])
                        add_dep_helper(wo_dma.ins, gate_mm[1].ins,
                                       sync=True, reason="defer wo load")
                elif not causal and lc == (NTC // B) - 1:
                    # non-causal needs the batch's full K/V first
                    for qc in range(4):
                        attn_chunk(b, qc)
                    flush_half(b, 0)
                    if b == 1:
                        flush_q11(0)
                        flush_q11(1)
                    else:
                        flush_half(b, 1)
                    if tci == 3:
                        wo_dma = nc.sync.dma_start(out=wo_sb[:],
                                                   in_=wop[:, :])
                        add_dep_helper(wo_dma.ins, gate_mm[1].ins,
                                       sync=True, reason="defer wo load")

            outproj(0, 0)
            outproj(0, 1)
            outproj(1, 0)
            outproj(1, 1, split=True)
            if dbg:
                nc.sync.dma_start(out=attnD[:, :], in_=attn_sb[:])
                nc.gpsimd.dma_start(out=agD[:, :], in_=ag[0][1][:, :])
                nc.sync.dma_start(out=qD[:, :], in_=q_sb[:])
                nc.sync.dma_start(out=kD[:, :], in_=k_sb[:])
                nc.sync.dma_start(out=vD[:, :], in_=vaug_sb[:])

    nc.compile()
    return nc


def _host_prep(inputs):
    x = np.ascontiguousarray(np.asarray(inputs["x"], np.float32).reshape(NT, D))
    wq = np.asarray(inputs["wq"], np.float32)
    wk = np.asarray(inputs["wk"], np.float32)
    wv = np.asarray(inputs["wv"], np.float32)
    wo = np.asarray(inputs["wo"], np.float32)
    cos = np.asarray(inputs["freqs_cos"], np.float32)
    sin = np.asarray(inputs["freqs_sin"], np.float32)
    mask = np.asarray(inputs["mask"], np.float32).reshape(S, S)

    tril = np.tril(np.ones((S, S), bool))
    if not mask.any():
        mode = "zeros"
    elif (mask[tril] == 0).all() and (mask[~tril] <= -1e8).all():
        mode = "causal"
    else:
        mode = "general"

    # x packed tci-major: xRe[p, (tci*KC+kk)*TCH + t] = x[tci*TCH+t, kk*P+p]
    xRe = np.ascontiguousarray(
        x.reshape(NTC, TCH, KC, P).transpose(3, 0, 2, 1)
        .reshape(P, NTC * KC * TCH).astype(BF))

    C = np.empty((P, S), np.float32)
    Sn = np.empty((P, S), np.float32)
    C[0::2] = cos.T
    C[1::2] = cos.T
    Sn[0::2] = -sin.T
    Sn[1::2] = sin.T
    cro = np.ascontiguousarray(C.astype(BF))
    sro = np.ascontiguousarray(Sn.astype(BF))
    cst = np.zeros((P, 3 * P), np.float32)
    pr = np.zeros((P, P), np.float32)
    idx = np.arange(0, P, 2)
    pr[idx, idx + 1] = 1.0
    pr[idx + 1, idx] = 1.0
    cst[:, 0:P] = pr
    cst[:, P:2 * P] = np.triu(np.ones((P, P), np.float32))
    cst[:, 2 * P:3 * P] = np.eye(P)
    cst = np.ascontiguousarray(cst.astype(BF))

    def wpack(w, r):
        wT = np.ascontiguousarray(w[r, :].T)
        return np.ascontiguousarray(
            wT.reshape(KC, P, DQ).transpose(1, 0, 2)
            .reshape(P, KC * DQ).astype(BF))

    in_maps = []
    for c in range(N_CORES):
        r = slice(c * DQ, (c + 1) * DQ)
        m = {
            "xRe": xRe,
            "wqp": wpack(wq, r),
            "wkp": wpack(wk, r),
            "wvp": wpack(wv, r),
            "wop": wpack(wo, r),
            "cro": cro,
            "sro": sro,
            "cst": cst,
        }
        if mode == "general":
            m["mskT"] = np.ascontiguousarray(
                (mask.T * math.sqrt(HD)).astype(BF))
        in_maps.append(m)
    return mode, in_maps


LAST_RESULT = None


def kernel(**inputs):
    global LAST_RESULT
    from concourse.bass_utils import run_bass_kernel_spmd

    mode, in_maps = _host_prep(inputs)
    if mode not in _cache:
        _cache[mode] = _build(mode)
    nc = _cache[mode]

    res = run_bass_kernel_spmd(nc, in_maps, list(range(N_CORES)))
    LAST_RESULT = res

    out_full = np.empty((NT, D), np.float32)
    for c in range(N_CORES):
        out_full[:, c * DQ:(c + 1) * DQ] = res.results[c]["outT"].T
    return out_full.reshape(B, S, D)
